# revision 19
# baseline (speedup 1.0000x reference)
"""MoE (8 experts, top-2, sigmoid router, SwiGLU + shared expert) on 8 TRN2 cores.

Strategy: expert-parallel with host-side dispatch. The router (x @ gate,
sigmoid, top-2) is cheap and runs on the host as part of sharding; each core
owns one expert and receives exactly the tokens routed to it (padded to a
uniform C so all cores run the same program), plus a 256-token shard of the
sequence for the replicated shared expert. This computes only the selected
top-2 expert branches instead of all 8, cutting matmul work ~3x versus dense.

On-device layout keeps tokens on the matmul *free* axis (weights stationary),
so up-projection, activation, and down-projection all happen without any
transposes; per-token routing scores are applied with DVE multiplies against a
host-prebroadcast [128, C] score tile (silu(s*g) * (s*u), matching the
reference's score-before-expert application). Weights are pre-swizzled on the
host into [128, blocks, cols] layout so each weight panel loads in a single
large DMA (the cost model charges ~625ns of serialized HWDGE time per DMA, so
few/large transfers matter). The host scatters per-expert outputs back into
the full sequence (indices within one expert are unique, so fancy-index add is
exact).
"""
import numpy as np
import ml_dtypes

import concourse.bass as bass  # noqa: F401  (imported for side effects/parity)
import concourse.tile as tile
from concourse import bacc, mybir
from concourse.bass_utils import run_bass_kernel_spmd

P = 128
N_CORES = 8
SLEN = 2048
DIM = 2048
HID = 1024
E = 8
SH = SLEN // N_CORES           # shared-expert tokens per core (256)
DC = DIM // P                  # 16 dim blocks
HC = HID // P                  # 8 hidden blocks
FD = 512                       # psum bank width (fp32) / panel width
BF16 = mybir.dt.bfloat16
F32 = mybir.dt.float32

_CACHE: dict = {}


def _chunks(C):
    n = -(-C // FD)
    sz = C // n                # C is rounded so n*8 divides it
    return [(i * sz, sz) for i in range(n)]


def _build(C):
    T = C + SH
    rch = _chunks(C)
    nc = bacc.Bacc("TRN2", target_bir_lowering=False, debug=False,
                   num_devices=N_CORES)

    xs_d = nc.dram_tensor("xs", [P, DC, T], BF16, kind="ExternalInput").ap()
    sb_d = nc.dram_tensor("sb", [P, C], F32, kind="ExternalInput").ap()
    w1e_d = nc.dram_tensor("w1e", [P, DC, HID], BF16, kind="ExternalInput").ap()
    w3e_d = nc.dram_tensor("w3e", [P, DC, HID], BF16, kind="ExternalInput").ap()
    w2e_d = nc.dram_tensor("w2e", [P, HC, DIM], BF16, kind="ExternalInput").ap()
    w1s_d = nc.dram_tensor("w1s", [P, DC, HID], BF16, kind="ExternalInput").ap()
    w3s_d = nc.dram_tensor("w3s", [P, DC, HID], BF16, kind="ExternalInput").ap()
    w2s_d = nc.dram_tensor("w2s", [P, HC, DIM], BF16, kind="ExternalInput").ap()
    y_d = nc.dram_tensor("y", [P, DC, T], F32, kind="ExternalOutput").ap()

    FP = 256                   # up-projection weight panel width
    with tile.TileContext(nc) as tc:
        with tc.tile_pool(name="const", bufs=1) as const_pool, \
             tc.tile_pool(name="wu", bufs=3) as wu_pool, \
             tc.tile_pool(name="w2p", bufs=4) as w2_pool, \
             tc.tile_pool(name="hp", bufs=1) as h_pool, \
             tc.tile_pool(name="actp", bufs=3) as act_pool, \
             tc.tile_pool(name="yop", bufs=3) as yo_pool, \
             tc.tile_pool(name="psA", bufs=2, space="PSUM") as psA, \
             tc.tile_pool(name="psY", bufs=3, space="PSUM") as psY:

            xs_sb = const_pool.tile([P, DC, T], BF16, tag="xs")
            sb_sb = const_pool.tile([P, C], F32, tag="sb")
            first = [True]
            zig = [0]



            for sec in range(2):           # 0 = routed expert, 1 = shared
                w1_src, w3_src, w2_src = (
                    (w1e_d, w3e_d, w2e_d) if sec == 0 else
                    (w1s_d, w3s_d, w2s_d))
                chs = rch if sec == 0 else [(0, SH)]
                base = 0 if sec == 0 else C
                secT = C if sec == 0 else SH

                # ---- up-projection: g/u for all hidden blocks ----
                h_sb = h_pool.tile([P, HC, secT], BF16, tag=f"h{sec}",
                                   name=f"h{sec}")
                for wp in range(HID // FP):            # 4 weight panels
                    w1h = wu_pool.tile([P, DC, FP], BF16, tag="w1h", name="w1h")
                    w3h = wu_pool.tile([P, DC, FP], BF16, tag="w3h", name="w3h")
                    if first[0]:
                        # Interleave quarter-panel weight loads with the x
                        # quarters (both routed chunks) so the PE can start a
                        # few us in instead of waiting ~19us for three full
                        # serialized transfers. sb rides after the first
                        # group; shared-x follows the second weight panel.
                        first[0] = False
                        c0 = chs[0][1]
                        for q in range(4):
                            sls = ([slice(0, 2), slice(2, 4)] if q == 0 else
                                   [slice(q * 4, (q + 1) * 4)])
                            for qs in sls:
                                nc.sync.dma_start(w1h[:, qs, :],
                                                  w1_src[:, qs, 0:FP])
                                nc.sync.dma_start(w3h[:, qs, :],
                                                  w3_src[:, qs, 0:FP])
                                nc.sync.dma_start(xs_sb[:, qs, 0:c0],
                                                  xs_d[:, qs, 0:c0])
                        nc.sync.dma_start(sb_sb[:], sb_d[:])
                        if C > c0:
                            for q in range(4):
                                qs = slice(q * 4, (q + 1) * 4)
                                nc.sync.dma_start(xs_sb[:, qs, c0:C],
                                                  xs_d[:, qs, c0:C])
                    else:
                        nc.sync.dma_start(w1h[:],
                                          w1_src[:, :, wp * FP:(wp + 1) * FP])
                        nc.sync.dma_start(w3h[:],
                                          w3_src[:, :, wp * FP:(wp + 1) * FP])
                        if sec == 0 and wp == 1:
                            # shared-x isn't needed until the shared section;
                            # keep it out of the startup-critical DMA stream
                            nc.sync.dma_start(xs_sb[:, :, C:T], xs_d[:, :, C:T])
                    def act_chain(pg, pu, hcg, ts, tn):
                        if sec == 0:
                            sg = act_pool.tile([P, FD], F32, tag="sg",
                                               name="sg")
                            nc.vector.tensor_mul(sg[:, :tn], pg[:, :tn],
                                                 sb_sb[:, ts:ts + tn])
                            ga = act_pool.tile([P, FD], BF16, tag="ga",
                                               name="ga")
                            nc.scalar.activation(
                                ga[:, :tn], sg[:, :tn],
                                mybir.ActivationFunctionType.Silu)
                            su = act_pool.tile([P, FD], BF16, tag="su",
                                               name="su")
                            nc.vector.tensor_mul(su[:, :tn], pu[:, :tn],
                                                 sb_sb[:, ts:ts + tn])
                            nc.vector.tensor_mul(
                                h_sb[:, hcg, ts:ts + tn], ga[:, :tn],
                                su[:, :tn])
                        else:
                            ga = act_pool.tile([P, FD], BF16, tag="ga",
                                               name="ga")
                            nc.scalar.activation(
                                ga[:, :tn], pg[:, :tn],
                                mybir.ActivationFunctionType.Silu)
                            nc.vector.tensor_mul(
                                h_sb[:, hcg, ts:ts + tn], ga[:, :tn],
                                pu[:, :tn])

                    if sec == 0 and wp == 0:
                        # First panel: keep both h4 accumulation groups live
                        # and sweep dc in quarter-groups, so each just-arrived
                        # DMA quarter feeds 2x the matmul work (the PE would
                        # otherwise outrun the serialized startup transfers).
                        for (ts, tn) in chs:
                            pgs = [psA.tile([P, FD], F32, tag="pg", name="pg")
                                   for _ in range(2)]
                            pus = [psA.tile([P, FD], F32, tag="pu", name="pu")
                                   for _ in range(2)]
                            for gq in range(4):
                                for h4 in range(FP // P):
                                    for dc in range(gq * 4, gq * 4 + 4):
                                        rhs = xs_sb[:, dc, ts:ts + tn]
                                        nc.tensor.matmul(
                                            pgs[h4][:, :tn],
                                            w1h[:, dc, h4 * P:(h4 + 1) * P],
                                            rhs, start=(dc == 0),
                                            stop=(dc == DC - 1))
                                        nc.tensor.matmul(
                                            pus[h4][:, :tn],
                                            w3h[:, dc, h4 * P:(h4 + 1) * P],
                                            rhs, start=(dc == 0),
                                            stop=(dc == DC - 1))
                            for h4 in range(FP // P):
                                act_chain(pgs[h4], pus[h4], h4, ts, tn)
                        continue

                    for (ts, tn) in chs:
                        for h4 in range(FP // P):      # 2 hid-128 blocks
                            hcg = wp * (FP // P) + h4
                            pg = psA.tile([P, FD], F32, tag="pg", name="pg")
                            pu = psA.tile([P, FD], F32, tag="pu", name="pu")
                            dcs = list(range(DC))
                            if zig[0] % 2:
                                dcs.reverse()
                            zig[0] += 1
                            for i, dc in enumerate(dcs):
                                rhs = xs_sb[:, dc, base + ts:base + ts + tn]
                                nc.tensor.matmul(
                                    pg[:, :tn], w1h[:, dc, h4 * P:(h4 + 1) * P],
                                    rhs, start=(i == 0), stop=(i == DC - 1))
                                nc.tensor.matmul(
                                    pu[:, :tn], w3h[:, dc, h4 * P:(h4 + 1) * P],
                                    rhs, start=(i == 0), stop=(i == DC - 1))
                            act_chain(pg, pu, hcg, ts, tn)

                # ---- down-projection ----
                for d4 in range(DIM // FD):            # 4 dim panels
                    w2c = w2_pool.tile([P, HC, FD], BF16, tag="w2c", name="w2c")
                    nc.sync.dma_start(w2c[:], w2_src[:, :, d4 * FD:(d4 + 1) * FD])
                    for ci, (ts, tn) in enumerate(chs):
                        tail = (sec == 1 and d4 == DIM // FD - 1
                                and ci == len(chs) - 1)
                        yo = yo_pool.tile([P, FD // P, FD], F32, tag="yo",
                                          name="yo")
                        for ds in range(FD // P):      # 4 dim-128 blocks
                            py = psY.tile([P, FD], F32, tag="py", name="py")
                            for hc in range(HC):
                                nc.tensor.matmul(
                                    py[:, :tn], w2c[:, hc, ds * P:(ds + 1) * P],
                                    h_sb[:, hc, ts:ts + tn],
                                    start=(hc == 0), stop=(hc == HC - 1))
                            nc.scalar.copy(yo[:, ds, :tn], py[:, :tn])
                            if tail:
                                # per-block writes so the kernel's last DMA is
                                # small and starts as soon as its copy lands
                                nc.sync.dma_start(
                                    y_d[:, d4 * (FD // P) + ds,
                                        base + ts:base + ts + tn],
                                    yo[:, ds, :tn])
                        if not tail:
                            nc.sync.dma_start(
                                y_d[:, d4 * (FD // P):(d4 + 1) * (FD // P),
                                    base + ts:base + ts + tn],
                                yo[:, :, :tn])

    nc.compile()
    return nc


def _get_nc():
    return _CACHE["nc"]


def _bf16(a):
    return np.ascontiguousarray(a.astype(ml_dtypes.bfloat16))


def _swz(mT, blocks):
    """[blocks*128, cols] -> [128, blocks, cols] (partition-major swizzle)."""
    r, cols = mT.shape
    assert r == blocks * P
    return np.ascontiguousarray(mT.reshape(blocks, P, cols).transpose(1, 0, 2))


def kernel(x, gate, expert_bias, w1, w2, w3, sw1, sw2, sw3):
    xt = np.asarray(x, np.float32).reshape(SLEN, DIM)
    gate = np.asarray(gate, np.float32)
    expert_bias = np.asarray(expert_bias, np.float32)

    # ---- router on host (part of the dispatch/sharding step) ----
    logits = xt @ gate
    scores = 1.0 / (1.0 + np.exp(-logits))
    biased = scores + expert_bias[None, :]
    order = np.argsort(-biased, axis=1, kind="stable")[:, :2]  # top-2, ties→low idx
    selmask = np.zeros((SLEN, E), bool)
    selmask[np.arange(SLEN), order[:, 0]] = True
    selmask[np.arange(SLEN), order[:, 1]] = True
    toks = [np.nonzero(selmask[:, e])[0] for e in range(E)]
    counts = [len(t) for t in toks]

    craw = max(max(counts), 1)
    nch = -(-craw // FD)
    C = -(-craw // (nch * 2)) * (nch * 2)  # divisible by nch, multiple of 2
    T = C + SH

    if _CACHE.get("C") != C:
        _CACHE["C"] = C
        _CACHE["nc"] = _build(C)
    nc = _CACHE["nc"]

    # ---- shared (replicated) tensors ----
    w1s = _bf16(_swz(np.asarray(sw1, np.float32).T, DC))
    w3s = _bf16(_swz(np.asarray(sw3, np.float32).T, DC))
    w2s = _bf16(_swz(np.asarray(sw2, np.float32).T, HC))

    in_maps = []
    for c in range(N_CORES):
        sel = toks[c]
        n = counts[c]
        xpack = np.zeros((T, DIM), np.float32)
        xpack[:n] = xt[sel]
        xpack[C:] = xt[c * SH:(c + 1) * SH]
        xs = _bf16(_swz(xpack.T, DC))                     # [128, 16, T]
        svec = np.zeros((C,), np.float32)
        svec[:n] = scores[sel, c]
        sb = np.ascontiguousarray(
            np.broadcast_to(svec[None, :], (P, C)).astype(np.float32))
        in_maps.append({
            "xs": xs, "sb": sb,
            "w1e": _bf16(_swz(np.asarray(w1[c], np.float32).T, DC)),
            "w3e": _bf16(_swz(np.asarray(w3[c], np.float32).T, DC)),
            "w2e": _bf16(_swz(np.asarray(w2[c], np.float32).T, HC)),
            "w1s": w1s, "w3s": w3s, "w2s": w2s,
        })

    res = run_bass_kernel_spmd(nc, in_maps, list(range(N_CORES)))

    out = np.empty((SLEN, DIM), np.float32)
    y2 = []
    for c in range(N_CORES):
        yv = np.asarray(res.results[c]["y"])              # [128, 16, T]
        y2c = np.ascontiguousarray(yv.transpose(1, 0, 2)).reshape(DIM, T)
        y2.append(y2c)
        out[c * SH:(c + 1) * SH] = y2c[:, C:].T           # shared expert part
    for c in range(N_CORES):
        n = counts[c]
        if n:
            out[toks[c]] += y2[c][:, :n].T                # routed part (unique idx)
    return out.reshape(1, 1, SLEN, DIM)


# revision 20
# speedup vs baseline: 1.0127x; 1.0127x over previous
"""MoE (8 experts, top-2, sigmoid router, SwiGLU + shared expert) on 8 TRN2 cores.

Strategy: expert-parallel with host-side dispatch. The router (x @ gate,
sigmoid, top-2) is cheap and runs on the host as part of sharding; each core
owns one expert and receives exactly the tokens routed to it (padded to a
uniform C so all cores run the same program), plus a 256-token shard of the
sequence for the replicated shared expert. This computes only the selected
top-2 expert branches instead of all 8, cutting matmul work ~3x versus dense.

On-device layout keeps tokens on the matmul *free* axis (weights stationary),
so up-projection, activation, and down-projection all happen without any
transposes; per-token routing scores are applied with DVE multiplies against a
host-prebroadcast [128, C] score tile (silu(s*g) * (s*u), matching the
reference's score-before-expert application). Weights are pre-swizzled on the
host into [128, blocks, cols] layout so each weight panel loads in a single
large DMA (the cost model charges ~625ns of serialized HWDGE time per DMA, so
few/large transfers matter). The host scatters per-expert outputs back into
the full sequence (indices within one expert are unique, so fancy-index add is
exact).
"""
import numpy as np
import ml_dtypes

import concourse.bass as bass  # noqa: F401  (imported for side effects/parity)
import concourse.tile as tile
from concourse import bacc, mybir
from concourse.bass_utils import run_bass_kernel_spmd

P = 128
N_CORES = 8
SLEN = 2048
DIM = 2048
HID = 1024
E = 8
SH = SLEN // N_CORES           # shared-expert tokens per core (256)
DC = DIM // P                  # 16 dim blocks
HC = HID // P                  # 8 hidden blocks
FD = 512                       # psum bank width (fp32) / panel width
BF16 = mybir.dt.bfloat16
F32 = mybir.dt.float32

_CACHE: dict = {}


def _chunks(C):
    n = -(-C // FD)
    sz = C // n                # C is rounded so n*8 divides it
    return [(i * sz, sz) for i in range(n)]


def _build(C):
    T = C + SH
    rch = _chunks(C)
    nc = bacc.Bacc("TRN2", target_bir_lowering=False, debug=False,
                   num_devices=N_CORES)

    xs_d = nc.dram_tensor("xs", [P, DC, T], BF16, kind="ExternalInput").ap()
    sb_d = nc.dram_tensor("sb", [P, C], F32, kind="ExternalInput").ap()
    w1e_d = nc.dram_tensor("w1e", [P, DC, HID], BF16, kind="ExternalInput").ap()
    w3e_d = nc.dram_tensor("w3e", [P, DC, HID], BF16, kind="ExternalInput").ap()
    w2e_d = nc.dram_tensor("w2e", [P, HC, DIM], BF16, kind="ExternalInput").ap()
    w1s_d = nc.dram_tensor("w1s", [P, DC, HID], BF16, kind="ExternalInput").ap()
    w3s_d = nc.dram_tensor("w3s", [P, DC, HID], BF16, kind="ExternalInput").ap()
    w2s_d = nc.dram_tensor("w2s", [P, HC, DIM], BF16, kind="ExternalInput").ap()
    y_d = nc.dram_tensor("y", [P, DC, T], F32, kind="ExternalOutput").ap()

    FP = 256                   # up-projection weight panel width
    with tile.TileContext(nc) as tc:
        with tc.tile_pool(name="const", bufs=1) as const_pool, \
             tc.tile_pool(name="wu", bufs=3) as wu_pool, \
             tc.tile_pool(name="w2p", bufs=4) as w2_pool, \
             tc.tile_pool(name="hp", bufs=1) as h_pool, \
             tc.tile_pool(name="actp", bufs=3) as act_pool, \
             tc.tile_pool(name="yop", bufs=3) as yo_pool, \
             tc.tile_pool(name="psA", bufs=2, space="PSUM") as psA, \
             tc.tile_pool(name="psY", bufs=3, space="PSUM") as psY:

            xs_sb = const_pool.tile([P, DC, T], BF16, tag="xs")
            sb_sb = const_pool.tile([P, C], F32, tag="sb")
            first = [True]
            zig = [0]



            for sec in range(2):           # 0 = routed expert, 1 = shared
                w1_src, w3_src, w2_src = (
                    (w1e_d, w3e_d, w2e_d) if sec == 0 else
                    (w1s_d, w3s_d, w2s_d))
                chs = rch if sec == 0 else [(0, SH)]
                base = 0 if sec == 0 else C
                secT = C if sec == 0 else SH

                # ---- up-projection: g/u for all hidden blocks ----
                h_sb = h_pool.tile([P, HC, secT], BF16, tag=f"h{sec}",
                                   name=f"h{sec}")
                for wp in range(HID // FP):            # 4 weight panels
                    w1h = wu_pool.tile([P, DC, FP], BF16, tag="w1h", name="w1h")
                    w3h = wu_pool.tile([P, DC, FP], BF16, tag="w3h", name="w3h")
                    if first[0]:
                        # Interleave quarter-panel weight loads with the x
                        # quarters (both routed chunks) so the PE can start a
                        # few us in instead of waiting ~19us for three full
                        # serialized transfers. sb rides after the first
                        # group; shared-x follows the second weight panel.
                        first[0] = False
                        c0 = chs[0][1]
                        for q in range(4):
                            qs = slice(q * 4, (q + 1) * 4)
                            nc.sync.dma_start(w1h[:, qs, :], w1_src[:, qs, 0:FP])
                            nc.sync.dma_start(w3h[:, qs, :], w3_src[:, qs, 0:FP])
                            nc.sync.dma_start(xs_sb[:, qs, 0:c0],
                                              xs_d[:, qs, 0:c0])
                        nc.sync.dma_start(sb_sb[:], sb_d[:])
                        if C > c0:
                            for q in range(4):
                                qs = slice(q * 4, (q + 1) * 4)
                                nc.sync.dma_start(xs_sb[:, qs, c0:C],
                                                  xs_d[:, qs, c0:C])
                    else:
                        nc.sync.dma_start(w1h[:],
                                          w1_src[:, :, wp * FP:(wp + 1) * FP])
                        nc.sync.dma_start(w3h[:],
                                          w3_src[:, :, wp * FP:(wp + 1) * FP])
                        if sec == 0 and wp == 1:
                            # shared-x isn't needed until the shared section;
                            # keep it out of the startup-critical DMA stream
                            nc.sync.dma_start(xs_sb[:, :, C:T], xs_d[:, :, C:T])
                    def act_chain(pg, pu, hcg, ts, tn):
                        if sec == 0:
                            sg = act_pool.tile([P, FD], F32, tag="sg",
                                               name="sg")
                            nc.vector.tensor_mul(sg[:, :tn], pg[:, :tn],
                                                 sb_sb[:, ts:ts + tn])
                            ga = act_pool.tile([P, FD], BF16, tag="ga",
                                               name="ga")
                            nc.scalar.activation(
                                ga[:, :tn], sg[:, :tn],
                                mybir.ActivationFunctionType.Silu)
                            su = act_pool.tile([P, FD], BF16, tag="su",
                                               name="su")
                            nc.vector.tensor_mul(su[:, :tn], pu[:, :tn],
                                                 sb_sb[:, ts:ts + tn])
                            nc.vector.tensor_mul(
                                h_sb[:, hcg, ts:ts + tn], ga[:, :tn],
                                su[:, :tn])
                        else:
                            ga = act_pool.tile([P, FD], BF16, tag="ga",
                                               name="ga")
                            nc.scalar.activation(
                                ga[:, :tn], pg[:, :tn],
                                mybir.ActivationFunctionType.Silu)
                            nc.vector.tensor_mul(
                                h_sb[:, hcg, ts:ts + tn], ga[:, :tn],
                                pu[:, :tn])

                    if sec == 0 and wp == 0:
                        # First panel: keep both h4 accumulation groups live
                        # and sweep dc in quarter-groups, so each just-arrived
                        # DMA quarter feeds 2x the matmul work (the PE would
                        # otherwise outrun the serialized startup transfers).
                        for (ts, tn) in chs:
                            pgs = [psA.tile([P, FD], F32, tag="pg", name="pg")
                                   for _ in range(2)]
                            pus = [psA.tile([P, FD], F32, tag="pu", name="pu")
                                   for _ in range(2)]
                            for gq in range(4):
                                for h4 in range(FP // P):
                                    for dc in range(gq * 4, gq * 4 + 4):
                                        rhs = xs_sb[:, dc, ts:ts + tn]
                                        nc.tensor.matmul(
                                            pgs[h4][:, :tn],
                                            w1h[:, dc, h4 * P:(h4 + 1) * P],
                                            rhs, start=(dc == 0),
                                            stop=(dc == DC - 1))
                                        nc.tensor.matmul(
                                            pus[h4][:, :tn],
                                            w3h[:, dc, h4 * P:(h4 + 1) * P],
                                            rhs, start=(dc == 0),
                                            stop=(dc == DC - 1))
                            for h4 in range(FP // P):
                                act_chain(pgs[h4], pus[h4], h4, ts, tn)
                        continue

                    for (ts, tn) in chs:
                        for h4 in range(FP // P):      # 2 hid-128 blocks
                            hcg = wp * (FP // P) + h4
                            pg = psA.tile([P, FD], F32, tag="pg", name="pg")
                            pu = psA.tile([P, FD], F32, tag="pu", name="pu")
                            dcs = list(range(DC))
                            if zig[0] % 2:
                                dcs.reverse()
                            zig[0] += 1
                            for i, dc in enumerate(dcs):
                                rhs = xs_sb[:, dc, base + ts:base + ts + tn]
                                nc.tensor.matmul(
                                    pg[:, :tn], w1h[:, dc, h4 * P:(h4 + 1) * P],
                                    rhs, start=(i == 0), stop=(i == DC - 1))
                                nc.tensor.matmul(
                                    pu[:, :tn], w3h[:, dc, h4 * P:(h4 + 1) * P],
                                    rhs, start=(i == 0), stop=(i == DC - 1))
                            act_chain(pg, pu, hcg, ts, tn)

                # ---- down-projection ----
                for d4 in range(DIM // FD):            # 4 dim panels
                    w2c = w2_pool.tile([P, HC, FD], BF16, tag="w2c", name="w2c")
                    nc.sync.dma_start(w2c[:], w2_src[:, :, d4 * FD:(d4 + 1) * FD])
                    for ci, (ts, tn) in enumerate(chs):
                        tail = (sec == 1 and d4 == DIM // FD - 1
                                and ci == len(chs) - 1)
                        yo = yo_pool.tile([P, FD // P, FD], F32, tag="yo",
                                          name="yo")
                        for ds in range(FD // P):      # 4 dim-128 blocks
                            py = psY.tile([P, FD], F32, tag="py", name="py")
                            for hc in range(HC):
                                nc.tensor.matmul(
                                    py[:, :tn], w2c[:, hc, ds * P:(ds + 1) * P],
                                    h_sb[:, hc, ts:ts + tn],
                                    start=(hc == 0), stop=(hc == HC - 1))
                            nc.scalar.copy(yo[:, ds, :tn], py[:, :tn])
                            if tail:
                                # per-block writes so the kernel's last DMA is
                                # small and starts as soon as its copy lands
                                nc.sync.dma_start(
                                    y_d[:, d4 * (FD // P) + ds,
                                        base + ts:base + ts + tn],
                                    yo[:, ds, :tn])
                        if not tail:
                            nc.sync.dma_start(
                                y_d[:, d4 * (FD // P):(d4 + 1) * (FD // P),
                                    base + ts:base + ts + tn],
                                yo[:, :, :tn])

    nc.compile()
    return nc


def _get_nc():
    return _CACHE["nc"]


def _bf16(a):
    return np.ascontiguousarray(a.astype(ml_dtypes.bfloat16))


def _swz(mT, blocks):
    """[blocks*128, cols] -> [128, blocks, cols] (partition-major swizzle)."""
    r, cols = mT.shape
    assert r == blocks * P
    return np.ascontiguousarray(mT.reshape(blocks, P, cols).transpose(1, 0, 2))


def kernel(x, gate, expert_bias, w1, w2, w3, sw1, sw2, sw3):
    xt = np.asarray(x, np.float32).reshape(SLEN, DIM)
    gate = np.asarray(gate, np.float32)
    expert_bias = np.asarray(expert_bias, np.float32)

    # ---- router on host (part of the dispatch/sharding step) ----
    logits = xt @ gate
    scores = 1.0 / (1.0 + np.exp(-logits))
    biased = scores + expert_bias[None, :]
    order = np.argsort(-biased, axis=1, kind="stable")[:, :2]  # top-2, ties→low idx
    selmask = np.zeros((SLEN, E), bool)
    selmask[np.arange(SLEN), order[:, 0]] = True
    selmask[np.arange(SLEN), order[:, 1]] = True
    toks = [np.nonzero(selmask[:, e])[0] for e in range(E)]
    counts = [len(t) for t in toks]

    craw = max(max(counts), 1)
    nch = -(-craw // FD)
    C = -(-craw // (nch * 2)) * (nch * 2)  # divisible by nch, multiple of 2
    T = C + SH

    if _CACHE.get("C") != C:
        _CACHE["C"] = C
        _CACHE["nc"] = _build(C)
    nc = _CACHE["nc"]

    # ---- shared (replicated) tensors ----
    w1s = _bf16(_swz(np.asarray(sw1, np.float32).T, DC))
    w3s = _bf16(_swz(np.asarray(sw3, np.float32).T, DC))
    w2s = _bf16(_swz(np.asarray(sw2, np.float32).T, HC))

    in_maps = []
    for c in range(N_CORES):
        sel = toks[c]
        n = counts[c]
        xpack = np.zeros((T, DIM), np.float32)
        xpack[:n] = xt[sel]
        xpack[C:] = xt[c * SH:(c + 1) * SH]
        xs = _bf16(_swz(xpack.T, DC))                     # [128, 16, T]
        svec = np.zeros((C,), np.float32)
        svec[:n] = scores[sel, c]
        sb = np.ascontiguousarray(
            np.broadcast_to(svec[None, :], (P, C)).astype(np.float32))
        in_maps.append({
            "xs": xs, "sb": sb,
            "w1e": _bf16(_swz(np.asarray(w1[c], np.float32).T, DC)),
            "w3e": _bf16(_swz(np.asarray(w3[c], np.float32).T, DC)),
            "w2e": _bf16(_swz(np.asarray(w2[c], np.float32).T, HC)),
            "w1s": w1s, "w3s": w3s, "w2s": w2s,
        })

    res = run_bass_kernel_spmd(nc, in_maps, list(range(N_CORES)))

    out = np.empty((SLEN, DIM), np.float32)
    y2 = []
    for c in range(N_CORES):
        yv = np.asarray(res.results[c]["y"])              # [128, 16, T]
        y2c = np.ascontiguousarray(yv.transpose(1, 0, 2)).reshape(DIM, T)
        y2.append(y2c)
        out[c * SH:(c + 1) * SH] = y2c[:, C:].T           # shared expert part
    for c in range(N_CORES):
        n = counts[c]
        if n:
            out[toks[c]] += y2[c][:, :n].T                # routed part (unique idx)
    return out.reshape(1, 1, SLEN, DIM)
